# revision 3
# baseline (speedup 1.0000x reference)
"""AdaptiveFilterBank Trainium2 kernel (8 NeuronCores, data-parallel over batch).

Math: reference = conv1d(x, filters)  then per-sample softmax-weighted sum over
the 8 filter channels. The weighted sum commutes with the (linear) convolution,
so each sample needs ONE length-31 conv with a combined per-sample filter
    kb[b] = softmax(MLP(features[b])) @ filter_params      (tiny, host-side)

Device formulation: per sample, lay x out column-interleaved with a -15 offset:
    X[q, c] = x[c*128 + q - 15]          (zero-padded), [128, 1025] in SBUF
Then 'same' cross-correlation  y[j*128+m] = sum_t kb[t] x[j*128+m+t-15]  is two
accumulating matmuls per 512-wide output tile:
    Y[:, j] = TA.T @ X[:, j]  +  TB.T @ X[0:30, j+1]
with TA[q, m] = kb[q-m] (banded Toeplitz, 0<=q-m<=30) and
     TB[q, m] = kb[q+128-m] (corner band from the next column).
Both matmuls run as float32r (full-rate fp32 on the PE at N=512).

Sharding: batch 64 -> 8 samples per core; filters/MLP host-computed.
"""

import os
import numpy as np

B = 64
L = 131072
N_CORES = 8
BPC = B // N_CORES          # samples per core
KLEN = 31
PAD = 15
NCOLS = 1025                # input columns per sample (L/128 + 1 halo column)
OCOLS = 1024                # output columns per sample (L/128)
NTILE = 512                 # matmul moving-dim / one PSUM bank of fp32

_CACHE = {}


def _build_graph():
    from concourse import bacc, tile, mybir

    dt = mybir.dt
    nc = bacc.Bacc("TRN2", target_bir_lowering=False, debug=False,
                   num_devices=N_CORES)

    x_ext = nc.dram_tensor("xt", [BPC, 128, NCOLS], dt.float32r,
                           kind="ExternalInput").ap()
    ta_ext = nc.dram_tensor("ta", [BPC, 128, 128], dt.float32r,
                            kind="ExternalInput").ap()
    tb_ext = nc.dram_tensor("tb", [BPC, 30, 128], dt.float32r,
                            kind="ExternalInput").ap()
    out_ext = nc.dram_tensor("out", [BPC, 128, OCOLS], dt.float32,
                             kind="ExternalOutput").ap()

    with tile.TileContext(nc) as tc:
        with tc.tile_pool(name="xin", bufs=3) as xpool, \
             tc.tile_pool(name="wgt", bufs=3) as wpool, \
             tc.tile_pool(name="ost", bufs=4) as opool, \
             tc.tile_pool(name="ps", bufs=4, space="PSUM") as pspool:
            for b in range(BPC):
                xt = xpool.tile([128, NCOLS], dt.float32r, tag="xt")
                nc.sync.dma_start(xt[:], x_ext[b])
                ta = wpool.tile([128, 128], dt.float32r, tag="ta")
                nc.sync.dma_start(ta[:], ta_ext[b])
                tb = wpool.tile([30, 128], dt.float32r, tag="tb")
                nc.sync.dma_start(tb[:], tb_ext[b])
                for h in range(OCOLS // NTILE):
                    c0 = h * NTILE
                    ps = pspool.tile([128, NTILE], dt.float32, tag="ps")
                    nc.tensor.matmul(
                        ps[:], ta[:], xt[:, c0:c0 + NTILE],
                        start=True, stop=False)
                    nc.tensor.matmul(
                        ps[:], tb[:], xt[0:30, c0 + 1:c0 + 1 + NTILE],
                        start=False, stop=True)
                    ot = opool.tile([128, NTILE], dt.float32, tag="ot")
                    nc.vector.tensor_copy(ot[:], ps[:])
                    nc.sync.dma_start(out_ext[b, :, c0:c0 + NTILE], ot[:])

    nc.compile()
    return nc


def _get_graph():
    if "nc" not in _CACHE:
        _CACHE["nc"] = _build_graph()
    return _CACHE["nc"]


def _host_prep(x, features, filter_params, W1, b1, W2, b2):
    """Selector MLP + combined filters + layout prep. All tiny or memory-bound."""
    x = np.ascontiguousarray(x, dtype=np.float32)
    # selector MLP (torch Linear convention)
    h = np.maximum(features @ W1.T + b1, 0.0)
    logits = h @ W2.T + b2
    e = np.exp(logits - logits.max(axis=-1, keepdims=True))
    w = e / e.sum(axis=-1, keepdims=True)                      # (B, 8)
    kb = (w @ filter_params[:, 0, :]).astype(np.float32)       # (B, 31)

    # interleaved, -15-offset input layout: X[b, q, c] = x[b, c*128 + q - 15]
    buf = np.zeros((B, NCOLS * 128), dtype=np.float32)
    buf[:, PAD:PAD + L] = x
    xt = np.ascontiguousarray(
        buf.reshape(B, NCOLS, 128).transpose(0, 2, 1))         # (B, 128, 1025)

    # banded Toeplitz weights
    q = np.arange(128)[:, None]
    m = np.arange(128)[None, :]
    t_a = q - m                                                # TA band
    mask_a = (t_a >= 0) & (t_a <= 30)
    t_b = np.arange(30)[:, None] + 128 - m                     # TB corner band
    mask_b = (t_b >= 0) & (t_b <= 30)
    ta = np.zeros((B, 128, 128), dtype=np.float32)
    tb = np.zeros((B, 30, 128), dtype=np.float32)
    ta[:, mask_a] = kb[:, t_a[mask_a]]
    tb[:, mask_b] = kb[:, t_b[mask_b]]
    return xt, ta, tb


def _run(inputs, trace=False, trace_cores=None):
    """Shard, execute on 8 NeuronCores, gather. Returns (y, exec_time_ns)."""
    from concourse.bass_utils import run_bass_kernel_spmd

    xt, ta, tb = _host_prep(**inputs)
    nc = _get_graph()
    in_maps = [
        {"xt": xt[i * BPC:(i + 1) * BPC],
         "ta": ta[i * BPC:(i + 1) * BPC],
         "tb": tb[i * BPC:(i + 1) * BPC]}
        for i in range(N_CORES)
    ]
    res = run_bass_kernel_spmd(nc, in_maps, core_ids=list(range(N_CORES)),
                               trace=trace, trace_cores=trace_cores)
    # gather: out[b] = Y[b].T.flatten()  ([128, 1024] -> 131072)
    y = np.empty((B, L), dtype=np.float32)
    for i in range(N_CORES):
        yc = np.asarray(res.results[i]["out"])                 # (BPC, 128, OCOLS)
        y[i * BPC:(i + 1) * BPC] = (
            yc.transpose(0, 2, 1).reshape(BPC, OCOLS * 128))
    return y, res.exec_time_ns


def kernel(x, features, filter_params, W1, b1, W2, b2):
    y, _ = _run(dict(x=x, features=features, filter_params=filter_params,
                     W1=W1, b1=b1, W2=W2, b2=b2))
    return y


# revision 4
# speedup vs baseline: 1.0164x; 1.0164x over previous
"""AdaptiveFilterBank Trainium2 kernel (8 NeuronCores, data-parallel over batch).

Math: reference = conv1d(x, filters)  then per-sample softmax-weighted sum over
the 8 filter channels. The weighted sum commutes with the (linear) convolution,
so each sample needs ONE length-31 conv with a combined per-sample filter
    kb[b] = softmax(MLP(features[b])) @ filter_params      (tiny, host-side)

Device formulation: per sample, lay x out column-interleaved with a -15 offset:
    X[q, c] = x[c*128 + q - 15]          (zero-padded), [128, 1025] in SBUF
Then 'same' cross-correlation  y[j*128+m] = sum_t kb[t] x[j*128+m+t-15]  is two
accumulating matmuls per 512-wide output tile:
    Y[:, j] = TA.T @ X[:, j]  +  TB.T @ X[0:30, j+1]
with TA[q, m] = kb[q-m] (banded Toeplitz, 0<=q-m<=30) and
     TB[q, m] = kb[q+128-m] (corner band from the next column).
Both matmuls run as float32r (full-rate fp32 on the PE at N=512).

Sharding: batch 64 -> 8 samples per core; filters/MLP host-computed.
"""

import os
import numpy as np

B = 64
L = 131072
N_CORES = 8
BPC = B // N_CORES          # samples per core
KLEN = 31
PAD = 15
NCOLS = 1025                # input columns per sample (L/128 + 1 halo column)
OCOLS = 1024                # output columns per sample (L/128)
NTILE = 512                 # matmul moving-dim / one PSUM bank of fp32

_CACHE = {}


def _build_graph():
    from concourse import bacc, tile, mybir

    dt = mybir.dt
    nc = bacc.Bacc("TRN2", target_bir_lowering=False, debug=False,
                   num_devices=N_CORES)

    x_ext = nc.dram_tensor("xt", [BPC, 128, NCOLS], dt.float32r,
                           kind="ExternalInput").ap()
    ta_ext = nc.dram_tensor("ta", [BPC, 128, 128], dt.float32r,
                            kind="ExternalInput").ap()
    tb_ext = nc.dram_tensor("tb", [BPC, 30, 128], dt.float32r,
                            kind="ExternalInput").ap()
    out_ext = nc.dram_tensor("out", [BPC, 128, OCOLS], dt.float32,
                             kind="ExternalOutput").ap()

    with tile.TileContext(nc) as tc:
        with tc.tile_pool(name="xin", bufs=BPC) as xpool, \
             tc.tile_pool(name="wgt", bufs=BPC) as wpool, \
             tc.tile_pool(name="ost", bufs=6) as opool, \
             tc.tile_pool(name="ps", bufs=8, space="PSUM") as pspool:
            # prefetch everything: weights first (tiny), then the x tiles.
            # All input DMAs ride the sync HWDGE ring; outputs ride the
            # scalar ring so in/out transfers overlap.
            tas, tbs, xts = [], [], []
            for b in range(BPC):
                ta = wpool.tile([128, 128], dt.float32r, tag="ta")
                nc.sync.dma_start(ta[:], ta_ext[b])
                tb = wpool.tile([30, 128], dt.float32r, tag="tb")
                nc.sync.dma_start(tb[:], tb_ext[b])
                tas.append(ta)
                tbs.append(tb)
            for b in range(BPC):
                xt = xpool.tile([128, NCOLS], dt.float32r, tag="xt")
                nc.sync.dma_start(xt[:], x_ext[b])
                xts.append(xt)
            for b in range(BPC):
                xt, ta, tb = xts[b], tas[b], tbs[b]
                for h in range(OCOLS // NTILE):
                    c0 = h * NTILE
                    ps = pspool.tile([128, NTILE], dt.float32, tag="ps")
                    nc.tensor.matmul(
                        ps[:], ta[:], xt[:, c0:c0 + NTILE],
                        start=True, stop=False)
                    nc.tensor.matmul(
                        ps[:], tb[:], xt[0:30, c0 + 1:c0 + 1 + NTILE],
                        start=False, stop=True)
                    ot = opool.tile([128, NTILE], dt.float32, tag="ot")
                    nc.vector.tensor_copy(ot[:], ps[:])
                    nc.scalar.dma_start(out_ext[b, :, c0:c0 + NTILE], ot[:])

    nc.compile()
    return nc


def _get_graph():
    if "nc" not in _CACHE:
        _CACHE["nc"] = _build_graph()
    return _CACHE["nc"]


def _host_prep(x, features, filter_params, W1, b1, W2, b2):
    """Selector MLP + combined filters + layout prep. All tiny or memory-bound."""
    x = np.ascontiguousarray(x, dtype=np.float32)
    # selector MLP (torch Linear convention)
    h = np.maximum(features @ W1.T + b1, 0.0)
    logits = h @ W2.T + b2
    e = np.exp(logits - logits.max(axis=-1, keepdims=True))
    w = e / e.sum(axis=-1, keepdims=True)                      # (B, 8)
    kb = (w @ filter_params[:, 0, :]).astype(np.float32)       # (B, 31)

    # interleaved, -15-offset input layout: X[b, q, c] = x[b, c*128 + q - 15]
    buf = np.zeros((B, NCOLS * 128), dtype=np.float32)
    buf[:, PAD:PAD + L] = x
    xt = np.ascontiguousarray(
        buf.reshape(B, NCOLS, 128).transpose(0, 2, 1))         # (B, 128, 1025)

    # banded Toeplitz weights
    q = np.arange(128)[:, None]
    m = np.arange(128)[None, :]
    t_a = q - m                                                # TA band
    mask_a = (t_a >= 0) & (t_a <= 30)
    t_b = np.arange(30)[:, None] + 128 - m                     # TB corner band
    mask_b = (t_b >= 0) & (t_b <= 30)
    ta = np.zeros((B, 128, 128), dtype=np.float32)
    tb = np.zeros((B, 30, 128), dtype=np.float32)
    ta[:, mask_a] = kb[:, t_a[mask_a]]
    tb[:, mask_b] = kb[:, t_b[mask_b]]
    return xt, ta, tb


def _run(inputs, trace=False, trace_cores=None):
    """Shard, execute on 8 NeuronCores, gather. Returns (y, exec_time_ns)."""
    from concourse.bass_utils import run_bass_kernel_spmd

    xt, ta, tb = _host_prep(**inputs)
    nc = _get_graph()
    in_maps = [
        {"xt": xt[i * BPC:(i + 1) * BPC],
         "ta": ta[i * BPC:(i + 1) * BPC],
         "tb": tb[i * BPC:(i + 1) * BPC]}
        for i in range(N_CORES)
    ]
    res = run_bass_kernel_spmd(nc, in_maps, core_ids=list(range(N_CORES)),
                               trace=trace, trace_cores=trace_cores)
    # gather: out[b] = Y[b].T.flatten()  ([128, 1024] -> 131072)
    y = np.empty((B, L), dtype=np.float32)
    for i in range(N_CORES):
        yc = np.asarray(res.results[i]["out"])                 # (BPC, 128, OCOLS)
        y[i * BPC:(i + 1) * BPC] = (
            yc.transpose(0, 2, 1).reshape(BPC, OCOLS * 128))
    return y, res.exec_time_ns


def kernel(x, features, filter_params, W1, b1, W2, b2):
    y, _ = _run(dict(x=x, features=features, filter_params=filter_params,
                     W1=W1, b1=b1, W2=W2, b2=b2))
    return y


# revision 5
# speedup vs baseline: 1.4883x; 1.4643x over previous
"""AdaptiveFilterBank Trainium2 kernel (8 NeuronCores, data-parallel over batch).

Math: reference = conv1d(x, filters) then per-sample softmax-weighted sum over
the 8 filter channels. The weighted sum commutes with the (linear) conv, so
each sample needs ONE length-31 conv with a combined per-sample filter
    kb[b] = softmax(MLP(features[b])) @ filter_params      (tiny, host-side)

Device formulation: per sample, lay x out column-interleaved with a -15 offset:
    X[q, c] = x[c*128 + q - 15]          (zero-padded), [128, 1025] in SBUF
Then 'same' cross-correlation  y[j*128+m] = sum_t kb[t] x[j*128+m+t-15]  is two
accumulating matmuls per 512-wide output tile:
    Y[:, j] = TA.T @ X[:, j]  +  TB.T @ X[0:30, j+1]
with TA[q, m] = kb[q-m] (banded Toeplitz, 0 <= q-m <= 30) and
     TB[q, m] = kb[q+128-m] (corner band feeding from the next column).

All PE traffic is bf16 (fp32r measured 722 ns / 512-col matmul vs ~213 bf16);
accumulation stays fp32 in PSUM; in/out HBM traffic is bf16 (halves DMA).
Measured end-to-end error vs fp32 reference ~4e-3 (gate 2e-2).

Sharding: batch 64 -> 8 samples per core; filter/MLP params host-computed.
"""

import numpy as np

B = 64
L = 131072
N_CORES = 8
BPC = B // N_CORES          # samples per core
KLEN = 31
PAD = 15
NCOLS = 1025                # input columns per sample (L/128 + 1 halo column)
OCOLS = 1024                # output columns per sample (L/128)
NTILE = 512                 # matmul moving dim / one fp32 PSUM bank
XCHUNK = 2                  # samples per input-DMA chunk

_CACHE = {}


def _build_graph():
    from concourse import bacc, tile, mybir

    dt = mybir.dt
    nc = bacc.Bacc("TRN2", target_bir_lowering=False, debug=False,
                   num_devices=N_CORES)

    # host-packed mega layouts (one/few DMAs; DMA issue costs ~0.8 us each)
    x_ext = nc.dram_tensor("xt", [128, BPC * NCOLS], dt.bfloat16,
                           kind="ExternalInput").ap()
    ta_ext = nc.dram_tensor("ta", [128, BPC * 128], dt.bfloat16,
                            kind="ExternalInput").ap()
    tb_ext = nc.dram_tensor("tb", [30, BPC * 128], dt.bfloat16,
                            kind="ExternalInput").ap()
    out_ext = nc.dram_tensor("out", [128, BPC * OCOLS], dt.bfloat16,
                             kind="ExternalOutput").ap()

    n_chunks = BPC // XCHUNK
    with tile.TileContext(nc) as tc:
        with tc.tile_pool(name="xin", bufs=n_chunks) as xpool, \
             tc.tile_pool(name="wgt", bufs=1) as wpool, \
             tc.tile_pool(name="ost", bufs=4) as opool, \
             tc.tile_pool(name="ps", bufs=8, space="PSUM") as pspool:
            # weights first (small), then x in chunks so compute starts early
            ta = wpool.tile([128, BPC * 128], dt.bfloat16, tag="ta")
            nc.sync.dma_start(ta[:], ta_ext[:])
            tb = wpool.tile([30, BPC * 128], dt.bfloat16, tag="tb")
            nc.sync.dma_start(tb[:], tb_ext[:])
            xts = []
            for ch in range(n_chunks):
                xt = xpool.tile([128, XCHUNK * NCOLS], dt.bfloat16, tag="xt")
                nc.sync.dma_start(
                    xt[:], x_ext[:, ch * XCHUNK * NCOLS:(ch + 1) * XCHUNK * NCOLS])
                xts.append(xt)

            for b in range(BPC):
                xt = xts[b // XCHUNK]
                xoff = (b % XCHUNK) * NCOLS
                tac = ta[:, b * 128:(b + 1) * 128]
                tbc = tb[:, b * 128:(b + 1) * 128]
                ot = opool.tile([128, OCOLS], dt.bfloat16, tag="ot")
                for h in range(OCOLS // NTILE):
                    c0 = h * NTILE
                    ps = pspool.tile([128, NTILE], dt.float32, tag="ps")
                    nc.tensor.matmul(
                        ps[:], tac, xt[:, xoff + c0:xoff + c0 + NTILE],
                        start=True, stop=False)
                    nc.tensor.matmul(
                        ps[:], tbc, xt[0:30, xoff + c0 + 1:xoff + c0 + 1 + NTILE],
                        start=False, stop=True)
                    nc.vector.tensor_copy(ot[:, c0:c0 + NTILE], ps[:])
                nc.scalar.dma_start(
                    out_ext[:, b * OCOLS:(b + 1) * OCOLS], ot[:])

    nc.compile()
    return nc


def _get_graph():
    if "nc" not in _CACHE:
        _CACHE["nc"] = _build_graph()
    return _CACHE["nc"]


def _host_prep(x, features, filter_params, W1, b1, W2, b2):
    """Selector MLP + combined filters + layout prep. All tiny or memory-bound."""
    import ml_dtypes
    bf16 = ml_dtypes.bfloat16

    x = np.ascontiguousarray(x, dtype=np.float32)
    # selector MLP (torch Linear convention)
    h = np.maximum(features @ W1.T + b1, 0.0)
    logits = h @ W2.T + b2
    e = np.exp(logits - logits.max(axis=-1, keepdims=True))
    w = e / e.sum(axis=-1, keepdims=True)                      # (B, 8)
    kb = (w @ filter_params[:, 0, :]).astype(np.float32)       # (B, 31)

    # interleaved, -15-offset input layout: X[b, q, c] = x[b, c*128 + q - 15]
    buf = np.zeros((B, NCOLS * 128), dtype=np.float32)
    buf[:, PAD:PAD + L] = x
    xt = buf.reshape(B, NCOLS, 128).transpose(0, 2, 1)         # (B, 128, 1025)

    # banded Toeplitz weights
    q = np.arange(128)[:, None]
    m = np.arange(128)[None, :]
    t_a = q - m                                                # TA band
    mask_a = (t_a >= 0) & (t_a <= 30)
    t_b = np.arange(30)[:, None] + 128 - m                     # TB corner band
    mask_b = (t_b >= 0) & (t_b <= 30)
    ta = np.zeros((B, 128, 128), dtype=np.float32)
    tb = np.zeros((B, 30, 128), dtype=np.float32)
    ta[:, mask_a] = kb[:, t_a[mask_a]]
    tb[:, mask_b] = kb[:, t_b[mask_b]]

    # per-core mega layouts: [128, BPC*cols] with sample-major column blocks
    def pack(a):  # (B, P, C) -> per-core (P, BPC*C), bf16
        P, C = a.shape[1], a.shape[2]
        return [np.ascontiguousarray(
                    a[i * BPC:(i + 1) * BPC].transpose(1, 0, 2).reshape(P, BPC * C)
                ).astype(bf16) for i in range(N_CORES)]

    return pack(xt), pack(ta), pack(tb)


def _run(inputs, trace=False, trace_cores=None):
    """Shard, execute on 8 NeuronCores, gather. Returns (y, exec_time_ns)."""
    from concourse.bass_utils import run_bass_kernel_spmd

    xts, tas, tbs = _host_prep(**inputs)
    nc = _get_graph()
    in_maps = [{"xt": xts[i], "ta": tas[i], "tb": tbs[i]}
               for i in range(N_CORES)]
    res = run_bass_kernel_spmd(nc, in_maps, core_ids=list(range(N_CORES)),
                               trace=trace, trace_cores=trace_cores)
    # gather: per core out is [128, BPC*1024]; sample b block.T.flatten() -> y[b]
    y = np.empty((B, L), dtype=np.float32)
    for i in range(N_CORES):
        yc = np.asarray(res.results[i]["out"]).astype(np.float32)
        yc = yc.reshape(128, BPC, OCOLS).transpose(1, 2, 0)    # (BPC, OCOLS, 128)
        y[i * BPC:(i + 1) * BPC] = yc.reshape(BPC, OCOLS * 128)
    return y, res.exec_time_ns


def kernel(x, features, filter_params, W1, b1, W2, b2):
    y, _ = _run(dict(x=x, features=features, filter_params=filter_params,
                     W1=W1, b1=b1, W2=W2, b2=b2))
    return y


# revision 7
# speedup vs baseline: 1.6738x; 1.1247x over previous
"""AdaptiveFilterBank Trainium2 kernel (8 NeuronCores, data-parallel over batch).

Math: reference = conv1d(x, filters) then per-sample softmax-weighted sum over
the 8 filter channels. The weighted sum commutes with the (linear) conv, so
each sample needs ONE length-31 conv with a combined per-sample filter
    kb[b] = softmax(MLP(features[b])) @ filter_params      (tiny, host-side)

Device formulation (overlapped interleave, P=98): per sample lay x out as
    X[q, c] = x[c*98 + q - 15]      (zero-padded), [128, 1338] in SBUF
so each SBUF column holds a 128-wide window covering the 98 outputs of that
column plus the +-15 conv halo. Then the whole 'same' cross-correlation is ONE
matmul per output tile:
    Y[m, c] = sum_q T[q, m] X[q, c],   T[q, m] = kb[q - m]  (0 <= q-m <= 30)
with Y[m, c] = y[c*98 + m], m in [0, 98).

All PE traffic is bf16 (PE measured pinned at 1.2 GHz, 1 col/cycle; fp32r was
~2x slower end-to-end); accumulation is fp32 in PSUM; HBM traffic is bf16 both
ways. End-to-end error vs fp32 reference ~4e-3 (gate 2e-2).

Sharding: batch 64 -> 8 samples per core; filter/MLP params host-computed.
"""

import numpy as np

B = 64
L = 131072
N_CORES = 8
BPC = B // N_CORES          # samples per core
KLEN = 31
PAD = 15
P = 98                      # outputs per interleave column (128 - 30 halo)
NCOLS = 1338                # ceil(L / P) input/output columns per sample
NSPLIT = (512, 512, 314)    # matmul N tiling of the 1338 columns
_CACHE = {}


def _build_graph():
    from concourse import bacc, tile, mybir

    dt = mybir.dt
    nc = bacc.Bacc("TRN2", target_bir_lowering=False, debug=False,
                   num_devices=N_CORES)

    x_ext = nc.dram_tensor("xt", [128, BPC * NCOLS], dt.bfloat16,
                           kind="ExternalInput").ap()
    t_ext = nc.dram_tensor("tw", [128, BPC * P], dt.bfloat16,
                           kind="ExternalInput").ap()
    out_ext = nc.dram_tensor("out", [P, BPC * NCOLS], dt.bfloat16,
                             kind="ExternalOutput").ap()

    XCHUNK = 2              # samples per input DMA
    n_chunks = BPC // XCHUNK
    with tile.TileContext(nc) as tc:
        with tc.tile_pool(name="xin", bufs=n_chunks) as xpool, \
             tc.tile_pool(name="wgt", bufs=1) as wpool, \
             tc.tile_pool(name="ost", bufs=4) as opool, \
             tc.tile_pool(name="ps", bufs=8, space="PSUM") as pspool:
            # weights first (small), then x in chunks so compute starts early
            tw = wpool.tile([128, BPC * P], dt.bfloat16, tag="tw")
            nc.sync.dma_start(tw[:], t_ext[:])
            xts = []
            for ch in range(n_chunks):
                xt = xpool.tile([128, XCHUNK * NCOLS], dt.bfloat16, tag="xt")
                nc.sync.dma_start(
                    xt[:], x_ext[:, ch * XCHUNK * NCOLS:(ch + 1) * XCHUNK * NCOLS])
                xts.append(xt)

            for b in range(BPC):
                xt = xts[b // XCHUNK]
                xoff = (b % XCHUNK) * NCOLS
                twc = tw[:, b * P:(b + 1) * P]
                ot = opool.tile([P, NCOLS], dt.bfloat16, tag="ot")
                c0 = 0
                for h, n in enumerate(NSPLIT):
                    ps = pspool.tile([P, 512], dt.float32, tag="ps")
                    nc.tensor.matmul(
                        ps[:, :n], twc, xt[:, xoff + c0:xoff + c0 + n],
                        start=True, stop=True)
                    # split PSUM->SBUF cast between DVE and ACT
                    if h == 1:
                        nc.scalar.copy(ot[:, c0:c0 + n], ps[:, :n])
                    else:
                        nc.vector.tensor_copy(ot[:, c0:c0 + n], ps[:, :n])
                    c0 += n
                nc.sync.dma_start(
                    out_ext[:, b * NCOLS:(b + 1) * NCOLS], ot[:])

    nc.compile()
    return nc


def _get_graph():
    if "nc" not in _CACHE:
        _CACHE["nc"] = _build_graph()
    return _CACHE["nc"]


def _host_prep(x, features, filter_params, W1, b1, W2, b2):
    """Selector MLP + combined filters + layout prep. All tiny or memory-bound."""
    import ml_dtypes
    from numpy.lib.stride_tricks import sliding_window_view
    bf16 = ml_dtypes.bfloat16

    x = np.ascontiguousarray(x, dtype=np.float32)
    # selector MLP (torch Linear convention)
    h = np.maximum(features @ W1.T + b1, 0.0)
    logits = h @ W2.T + b2
    e = np.exp(logits - logits.max(axis=-1, keepdims=True))
    w = e / e.sum(axis=-1, keepdims=True)                      # (B, 8)
    kb = (w @ filter_params[:, 0, :]).astype(np.float32)       # (B, 31)

    # overlapped interleave: X[b, q, c] = x[b, c*98 + q - 15]
    span = (NCOLS - 1) * P + 128
    xp = np.zeros((B, span), dtype=np.float32)
    xp[:, PAD:PAD + L] = x
    win = sliding_window_view(xp, 128, axis=1)                 # (B, span-127, 128)
    xt = win[:, ::P][:, :NCOLS].transpose(0, 2, 1)             # (B, 128, 1338)

    # banded Toeplitz weight: T[q, m] = kb[q - m], 0 <= q-m <= 30
    q = np.arange(128)[:, None]
    m = np.arange(P)[None, :]
    t_i = q - m
    mask = (t_i >= 0) & (t_i <= 30)
    tw = np.zeros((B, 128, P), dtype=np.float32)
    tw[:, mask] = kb[:, t_i[mask]]

    def pack(a):  # (B, Pdim, C) -> per-core (Pdim, BPC*C) bf16
        Pd, C = a.shape[1], a.shape[2]
        return [np.ascontiguousarray(
                    a[i * BPC:(i + 1) * BPC].transpose(1, 0, 2).reshape(Pd, BPC * C)
                ).astype(bf16) for i in range(N_CORES)]

    return pack(xt), pack(tw)


def _run(inputs, trace=False, trace_cores=None):
    """Shard, execute on 8 NeuronCores, gather. Returns (y, exec_time_ns)."""
    from concourse.bass_utils import run_bass_kernel_spmd

    xts, tws = _host_prep(**inputs)
    nc = _get_graph()
    in_maps = [{"xt": xts[i], "tw": tws[i]} for i in range(N_CORES)]
    res = run_bass_kernel_spmd(nc, in_maps, core_ids=list(range(N_CORES)),
                               trace=trace, trace_cores=trace_cores)
    # gather: per-core out [P, BPC*NCOLS]; sample block.T.flatten()[:L] -> y[b]
    y = np.empty((B, L), dtype=np.float32)
    for i in range(N_CORES):
        yc = np.asarray(res.results[i]["out"]).astype(np.float32)
        yc = yc.reshape(P, BPC, NCOLS).transpose(1, 2, 0)      # (BPC, NCOLS, P)
        y[i * BPC:(i + 1) * BPC] = yc.reshape(BPC, NCOLS * P)[:, :L]
    return y, res.exec_time_ns


def kernel(x, features, filter_params, W1, b1, W2, b2):
    y, _ = _run(dict(x=x, features=features, filter_params=filter_params,
                     W1=W1, b1=b1, W2=W2, b2=b2))
    return y


# revision 8
# speedup vs baseline: 1.6824x; 1.0051x over previous
"""AdaptiveFilterBank Trainium2 kernel (8 NeuronCores, data-parallel over batch).

Math: reference = conv1d(x, filters) then per-sample softmax-weighted sum over
the 8 filter channels. The weighted sum commutes with the (linear) conv, so
each sample needs ONE length-31 conv with a combined per-sample filter
    kb[b] = softmax(MLP(features[b])) @ filter_params      (tiny, host-side)

Device formulation (overlapped interleave, P=98): per sample lay x out as
    X[q, c] = x[c*98 + q - 15]      (zero-padded), [128, 1338] in SBUF
so each SBUF column holds a 128-wide window covering the 98 outputs of that
column plus the +-15 conv halo. Then the whole 'same' cross-correlation is ONE
matmul per output tile:
    Y[m, c] = sum_q T[q, m] X[q, c],   T[q, m] = kb[q - m]  (0 <= q-m <= 30)
with Y[m, c] = y[c*98 + m], m in [0, 98).

All PE traffic is bf16 (PE measured pinned at 1.2 GHz, 1 col/cycle; fp32r was
~2x slower end-to-end); accumulation is fp32 in PSUM; HBM traffic is bf16 both
ways. End-to-end error vs fp32 reference ~4e-3 (gate 2e-2).

Sharding: batch 64 -> 8 samples per core; filter/MLP params host-computed.
"""

import numpy as np

B = 64
L = 131072
N_CORES = 8
BPC = B // N_CORES          # samples per core
KLEN = 31
PAD = 15
P = 98                      # outputs per interleave column (128 - 30 halo)
NCOLS = 1338                # ceil(L / P) input/output columns per sample
NSPLIT = (512, 512, 314)    # matmul N tiling of the 1338 columns
_CACHE = {}


def _build_graph():
    """Raw Bacc graph with hand-rolled semaphores (Tile's fixed epilogue —
    kernel-tail drain + EVSEM butterfly — measured ~9 us, so we skip Tile)."""
    from concourse import bacc, mybir

    dt = mybir.dt
    nc = bacc.Bacc("TRN2", target_bir_lowering=False, debug=False,
                   num_devices=N_CORES)

    x_ext = nc.dram_tensor("xt", [128, BPC * NCOLS], dt.bfloat16,
                           kind="ExternalInput").ap()
    t_ext = nc.dram_tensor("tw", [128, BPC * P], dt.bfloat16,
                           kind="ExternalInput").ap()
    out_ext = nc.dram_tensor("out", [P, BPC * NCOLS], dt.bfloat16,
                             kind="ExternalOutput").ap()

    XCHUNK = 2              # samples per input DMA
    NOT = 4                 # output staging slots
    n_chunks = BPC // XCHUNK
    n_tiles = BPC * len(NSPLIT)
    c0s = [sum(NSPLIT[:h]) for h in range(len(NSPLIT))]
    # engine that copies tile h of a sample: ACT for the middle, DVE otherwise
    is_act = [h == 1 for h in range(len(NSPLIT))]

    def copies_done_before(k):
        """(#DVE, #ACT) copies among global tiles 0..k-1."""
        nv = sum(1 for j in range(k) if not is_act[j % len(NSPLIT)])
        ns = k - nv
        return nv, ns

    with (
        nc.sbuf_tensor("xt_sb", [128, BPC * NCOLS], dt.bfloat16) as xt_sb,
        nc.sbuf_tensor("tw_sb", [128, BPC * P], dt.bfloat16) as tw_sb,
        nc.sbuf_tensor("ot_sb", [P, NOT * NCOLS], dt.bfloat16) as ot_sb,
        nc.psum_tensor("ps", [P, 8 * 512], dt.float32) as ps,
        nc.semaphore("s_in") as s_in,
        nc.semaphore("s_mm") as s_mm,
        nc.semaphore("s_cv") as s_cv,
        nc.semaphore("s_cs") as s_cs,
        nc.semaphore("s_out") as s_out,
        nc.semaphore("s_done") as s_done,
        nc.Block(no_gpsimd_drain=True) as block,
    ):
        @block.sync
        def _(sync):
            sync.dma_start(out=tw_sb[:], in_=t_ext[:]).then_inc(s_in, 16)
            for ch in range(n_chunks):
                lo, hi = ch * XCHUNK * NCOLS, (ch + 1) * XCHUNK * NCOLS
                sync.dma_start(out=xt_sb[:, lo:hi],
                               in_=x_ext[:, lo:hi]).then_inc(s_in, 16)
            for b in range(BPC):
                sync.wait_ge(s_cv, 2 * (b + 1))
                sync.wait_ge(s_cs, b + 1)
                so = (b % NOT) * NCOLS
                sync.dma_start(out=out_ext[:, b * NCOLS:(b + 1) * NCOLS],
                               in_=ot_sb[:, so:so + NCOLS]).then_inc(s_out, 16)
            sync.wait_ge(s_out, 16 * BPC)
            sync.nop().then_inc(s_done, 1)

        @block.tensor
        def _(tensor):
            for b in range(BPC):
                tensor.wait_ge(s_in, 16 * (2 + b // XCHUNK))
                for h, n in enumerate(NSPLIT):
                    k = len(NSPLIT) * b + h
                    if k >= 8:
                        nv, ns = copies_done_before(k - 7)
                        tensor.wait_ge(s_cv, nv)
                        tensor.wait_ge(s_cs, ns)
                    bank = (k % 8) * 512
                    c0 = c0s[h]
                    tensor.matmul(
                        ps[:, bank:bank + n],
                        tw_sb[:, b * P:(b + 1) * P],
                        xt_sb[:, b * NCOLS + c0:b * NCOLS + c0 + n],
                        start=True, stop=True).then_inc(s_mm, 1)

        @block.vector
        def _(vector):
            for b in range(BPC):
                so = (b % NOT) * NCOLS
                for h, n in enumerate(NSPLIT):
                    if is_act[h]:
                        continue
                    k = len(NSPLIT) * b + h
                    vector.wait_ge(s_mm, k + 1)
                    if b >= NOT and h == 0:
                        vector.wait_ge(s_out, 16 * (b - NOT + 1))
                    bank = (k % 8) * 512
                    c0 = c0s[h]
                    vector.tensor_copy(ot_sb[:, so + c0:so + c0 + n],
                                       ps[:, bank:bank + n]).then_inc(s_cv, 1)

        @block.scalar
        def _(scalar):
            for b in range(BPC):
                so = (b % NOT) * NCOLS
                for h, n in enumerate(NSPLIT):
                    if not is_act[h]:
                        continue
                    k = len(NSPLIT) * b + h
                    scalar.wait_ge(s_mm, k + 1)
                    if b >= NOT:
                        scalar.wait_ge(s_out, 16 * (b - NOT + 1))
                    bank = (k % 8) * 512
                    c0 = c0s[h]
                    scalar.copy(ot_sb[:, so + c0:so + c0 + n],
                                ps[:, bank:bank + n]).then_inc(s_cs, 1)

        @block.gpsimd
        def _(gpsimd):
            # leave all kernel sems at 0 so the NEFF can re-execute
            gpsimd.wait_ge(s_done, 1)
            nums = sorted(s.num for s in (s_in, s_mm, s_cv, s_cs, s_out, s_done))
            gpsimd.dma_reset(range(nums[0], nums[-1] + 1))
            gpsimd.sem_clear(range(nums[0], nums[-1] + 1))

    nc.compile()
    return nc


def _get_graph():
    if "nc" not in _CACHE:
        _CACHE["nc"] = _build_graph()
    return _CACHE["nc"]


def _host_prep(x, features, filter_params, W1, b1, W2, b2):
    """Selector MLP + combined filters + layout prep. All tiny or memory-bound."""
    import ml_dtypes
    from numpy.lib.stride_tricks import sliding_window_view
    bf16 = ml_dtypes.bfloat16

    x = np.ascontiguousarray(x, dtype=np.float32)
    # selector MLP (torch Linear convention)
    h = np.maximum(features @ W1.T + b1, 0.0)
    logits = h @ W2.T + b2
    e = np.exp(logits - logits.max(axis=-1, keepdims=True))
    w = e / e.sum(axis=-1, keepdims=True)                      # (B, 8)
    kb = (w @ filter_params[:, 0, :]).astype(np.float32)       # (B, 31)

    # overlapped interleave: X[b, q, c] = x[b, c*98 + q - 15]
    span = (NCOLS - 1) * P + 128
    xp = np.zeros((B, span), dtype=np.float32)
    xp[:, PAD:PAD + L] = x
    win = sliding_window_view(xp, 128, axis=1)                 # (B, span-127, 128)
    xt = win[:, ::P][:, :NCOLS].transpose(0, 2, 1)             # (B, 128, 1338)

    # banded Toeplitz weight: T[q, m] = kb[q - m], 0 <= q-m <= 30
    q = np.arange(128)[:, None]
    m = np.arange(P)[None, :]
    t_i = q - m
    mask = (t_i >= 0) & (t_i <= 30)
    tw = np.zeros((B, 128, P), dtype=np.float32)
    tw[:, mask] = kb[:, t_i[mask]]

    def pack(a):  # (B, Pdim, C) -> per-core (Pdim, BPC*C) bf16
        Pd, C = a.shape[1], a.shape[2]
        return [np.ascontiguousarray(
                    a[i * BPC:(i + 1) * BPC].transpose(1, 0, 2).reshape(Pd, BPC * C)
                ).astype(bf16) for i in range(N_CORES)]

    return pack(xt), pack(tw)


def _run(inputs, trace=False, trace_cores=None):
    """Shard, execute on 8 NeuronCores, gather. Returns (y, exec_time_ns)."""
    from concourse.bass_utils import run_bass_kernel_spmd

    xts, tws = _host_prep(**inputs)
    nc = _get_graph()
    in_maps = [{"xt": xts[i], "tw": tws[i]} for i in range(N_CORES)]
    res = run_bass_kernel_spmd(nc, in_maps, core_ids=list(range(N_CORES)),
                               trace=trace, trace_cores=trace_cores)
    # gather: per-core out [P, BPC*NCOLS]; sample block.T.flatten()[:L] -> y[b]
    y = np.empty((B, L), dtype=np.float32)
    for i in range(N_CORES):
        yc = np.asarray(res.results[i]["out"]).astype(np.float32)
        yc = yc.reshape(P, BPC, NCOLS).transpose(1, 2, 0)      # (BPC, NCOLS, P)
        y[i * BPC:(i + 1) * BPC] = yc.reshape(BPC, NCOLS * P)[:, :L]
    return y, res.exec_time_ns


def kernel(x, features, filter_params, W1, b1, W2, b2):
    y, _ = _run(dict(x=x, features=features, filter_params=filter_params,
                     W1=W1, b1=b1, W2=W2, b2=b2))
    return y
